# revision 5
# baseline (speedup 1.0000x reference)
"""Trainium2 Bass kernel for nn_ComplexSuperposition.

Math (per batch b):
    or = sum_t w[b,t] * x_r[b,t,:]          # [D]
    oi = sum_t w[b,t] * x_i[b,t,:]          # [D]
    out_r[b] = or (x) or + oi (x) oi        # [D,D]
    out_i[b] = oi (x) or - or (x) oi        # [D,D]

Strategy: pure data-parallel over B=128 across 8 cores (16 batches/core),
processed in pairs of batches. fp16 operands + fp16 DRAM outputs (upcast
on host) + block-upper-triangle outputs (out_r symmetric, out_i
antisymmetric; PE products are exactly mirror-consistent, so the host
mirror adds zero error).

  Phase A: weighted sums as K=128 matmuls with a host-precomputed one-hot
           stationary layout `wx`: for each pair of batches, 8 matmuls
           accumulate (or,oi) into PSUM rows 0-1 (even batch) / 32-33 (odd
           batch) of bank 0 and (oi,-or) into bank 1 of one 2-bank tile.
           MMs are ordered so shared-lhsT pairs are adjacent (6 LDWEIGHTS
           instead of 8) and banks alternate; one cast-copy evacuates all
           four operand pairs to SBUF fp16.
  Phase B: rank-2 outer products out = lhsT.T @ rhs with K=2 operands,
           reusing each stationary lhsT=mv for two matmuls (rhs=mv ->
           out_r, rhs=st -> -out_i; the host folds the sign into the
           triangle expansion). Even batches use PE row group 0, odd
           batches row group 1 (tile_position 32) so LDWEIGHTS overlaps
           in-flight matmuls. TRIANGLE mode computes chunk m over columns
           [128m, 512) only.
  IO:      both batches' real+imag planes arrive in one SWDGE DMA from a
           host-interleaved tensor; outputs leave in two half-width DMAs
           per tensor per pair (cols 0:896 after chunk 1, 896:1280 after
           chunk 3), out_r on the sync queue and out_i on the scalar
           queue. A burst of tiny warmup matmuls (enabled by an early
           gpsimd memset) locks the PE HAM clock gate at 2.4 GHz during
           the load prologue.
"""

import os
from contextlib import ExitStack

import numpy as np

N_CORES = 8
B, T, D = 128, 128, 512
B_LOC = B // N_CORES  # 16

# precision mode:
#   "fp16o" = fp16 operands AND fp16 DRAM outputs (upcast to fp32 on host)
#   "fp16"  = fp16 operands, fp32 outputs
PRECISION = os.environ.get("CS_PRECISION", "fp16o")
# triangle mode: device computes only the block-upper triangle of each
# [D,D] output (out_r symmetric, out_i antisymmetric), host mirrors the
# rest. The device's "i" plane holds -out_i; the host negates.
TRIANGLE = os.environ.get("CS_TRIANGLE", "1") == "1"
TRI_OFF = (0, 512, 896, 1152)  # free-dim offset of chunk m in packed row
TRI_W = 1280
HALF_COL = 896  # output DMA split point (end of chunk 1)

_CACHE = {}


def _build_program():
    import concourse.bacc as bacc
    import concourse.tile as tile
    from concourse import mybir

    f32 = mybir.dt.float32
    f16 = mybir.dt.float16
    dt_a = f16 if PRECISION in ("fp16o", "fp16") else f32
    dt_b = dt_a
    dt_o = f16 if PRECISION == "fp16o" else f32

    nc = bacc.Bacc("TRN2", target_bir_lowering=False, debug=False)

    xc_d = nc.dram_tensor("xcat", [B_LOC, T, 2, D], dt_a, kind="ExternalInput").ap()
    wx_d = nc.dram_tensor("wx", [T, 54 * B_LOC], dt_a, kind="ExternalInput").ap()
    if TRIANGLE:
        or_d = nc.dram_tensor("out_r", [B_LOC, 128, TRI_W], dt_o, kind="ExternalOutput").ap()
        oi_d = nc.dram_tensor("out_i", [B_LOC, 128, TRI_W], dt_o, kind="ExternalOutput").ap()
    else:
        or_d = nc.dram_tensor("out_r", [B_LOC, D, D], dt_o, kind="ExternalOutput").ap()
        oi_d = nc.dram_tensor("out_i", [B_LOC, D, D], dt_o, kind="ExternalOutput").ap()

    with tile.TileContext(nc) as tc, ExitStack() as ctx:
        singles = ctx.enter_context(tc.tile_pool(name="singles", bufs=1))
        xpool = ctx.enter_context(tc.tile_pool(name="x", bufs=8))
        vpool = ctx.enter_context(tc.tile_pool(name="vec", bufs=8))
        opool = ctx.enter_context(tc.tile_pool(name="outs", bufs=8))
        psa = ctx.enter_context(tc.tile_pool(name="psa", bufs=1, space="PSUM"))
        psb = ctx.enter_context(tc.tile_pool(name="psb", bufs=3, space="PSUM"))

        # PE warmup: the memset runs on the otherwise-idle vector engine so
        # the tiny matmul burst (~3.4us) starts immediately and spans the
        # load prologue, bringing the HAM clock gate to 8/8 before phase A.
        warm = singles.tile([2, 64], dt_b)
        nc.vector.memset(warm[:], 0)

        wx = singles.tile([T, 54 * B_LOC], dt_a)
        nc.sync.dma_start(out=wx[:], in_=wx_d[:])

        # pair 0's inputs ride the two HWDGE queues (one batch each) so
        # phase A can start ~4us in; pairs 1-7 load via gpsimd SWDGE.
        x4_first = xpool.tile([T, 2, 2, D], dt_a, tag="x")
        nc.sync.dma_start(out=x4_first[:, 0], in_=xc_d[0])
        nc.scalar.dma_start(out=x4_first[:, 1], in_=xc_d[1])

        wps = psa.tile([34, 2, D], f32, tag="pa")
        for _ in range(64):
            nc.tensor.matmul(wps[:32, 0, :64], lhsT=warm[:, :32], rhs=warm[:], start=True, stop=True)

        mm = nc.tensor.matmul
        for p in range(B_LOC // 2):
            c0 = 2 * p
            be = 108 * p       # even-batch wx block (width 6, pairs at rows 0-1)
            bo = 108 * p + 6   # odd-batch wx block (3x34, pairs at rows 32-33)

            if p == 0:
                x4 = x4_first
            else:
                x4 = xpool.tile([T, 2, 2, D], dt_a, tag="x")
                nc.gpsimd.dma_start(out=x4[:], in_=xc_d[c0 : c0 + 2].rearrange("j t r d -> t j r d"))
            xr0, xi0 = x4[:, 0, 0, :], x4[:, 0, 1, :]
            xr1, xi1 = x4[:, 1, 0, :], x4[:, 1, 1, :]

            # Phase A into one 2-bank pair tile shared by both batches:
            # bank j=0 rows (0,1,32,33) = (or_e, oi_e, or_o, oi_o)  [mv]
            # bank j=1 rows (0,1,32,33) = (oi_e, -or_e, oi_o, -or_o) [st]
            # Ordered so shared-lhsT MMs are adjacent and banks alternate.
            pa = psa.tile([34, 2, D], f32, tag="pa")
            mm(pa[:, 0, :], lhsT=wx[:, bo : bo + 34], rhs=xr1, start=True, stop=False, skip_group_check=True)
            mm(pa[:, 1, :], lhsT=wx[:, bo : bo + 34], rhs=xi1, start=True, stop=False, skip_group_check=True)
            mm(pa[:2, 0, :], lhsT=wx[:, be : be + 2], rhs=xr0, start=False, stop=False, skip_group_check=True)
            mm(pa[:2, 1, :], lhsT=wx[:, be : be + 2], rhs=xi0, start=False, stop=False, skip_group_check=True)
            mm(pa[:, 0, :], lhsT=wx[:, bo + 34 : bo + 68], rhs=xi1, start=False, stop=False, skip_group_check=True)
            mm(pa[:, 1, :], lhsT=wx[:, bo + 68 : bo + 102], rhs=xr1, start=False, stop=False, skip_group_check=True)
            mm(pa[:2, 0, :], lhsT=wx[:, be + 2 : be + 4], rhs=xi0, start=False, stop=True, skip_group_check=True)
            mm(pa[:2, 1, :], lhsT=wx[:, be + 4 : be + 6], rhs=xr0, start=False, stop=True, skip_group_check=True)

            # One evacuation for all four operand pairs
            mvst = vpool.tile([34, 2, D], dt_b, tag="op")
            if p % 2 == 0:
                nc.vector.tensor_copy(out=mvst[:], in_=pa[:])
            else:
                nc.scalar.copy(out=mvst[:], in_=pa[:])
            mv0, st0 = mvst[0:2, 0, :], mvst[0:2, 1, :]
            mv1, st1 = mvst[32:34, 0, :], mvst[32:34, 1, :]

            # Phase B: each lhsT serves two matmuls (rhs=mv -> out_r,
            # rhs=st -> -out_i); even/odd batches alternate PE row groups.
            ow = TRI_W if TRIANGLE else 4 * D
            big01 = opool.tile([128, 4, ow], dt_o, tag="big")  # planes: r_e, -i_e, r_o, -i_o
            bgr = big01[:].rearrange("p (b j) n -> p b j n", j=2)
            for m in range(4):
                msl = slice(m * 128, (m + 1) * 128)
                if TRIANGLE:
                    nsl = slice(m * 128, D)
                    nw = D - m * 128
                    oo = TRI_OFF[m]
                else:
                    nsl = slice(0, D)
                    nw = D
                    oo = m * D
                pp0 = psb.tile([128, 2, D], f32, tag="pb")
                pp1 = psb.tile([128, 2, D], f32, tag="pb")
                mm(pp0[:, 0, :nw], lhsT=mv0[:, msl], rhs=mv0[:, nsl], start=True, stop=True)
                mm(pp0[:, 1, :nw], lhsT=mv0[:, msl], rhs=st0[:, nsl], start=True, stop=True)
                mm(pp1[:, 0, :nw], lhsT=mv1[:, msl], rhs=mv1[:, nsl], start=True, stop=True)
                mm(pp1[:, 1, :nw], lhsT=mv1[:, msl], rhs=st1[:, nsl], start=True, stop=True)
                nc.vector.tensor_copy(out=big01[:, 0:2, oo : oo + nw], in_=pp0[:, :, :nw])
                nc.scalar.copy(out=big01[:, 2:4, oo : oo + nw], in_=pp1[:, :, :nw])

                if TRIANGLE and p == B_LOC // 2 - 1:
                    # last pair: per-chunk stores so the drain tail is only
                    # the final 128-col chunk
                    nc.sync.dma_start(
                        out=or_d[c0 : c0 + 2, :, oo : oo + nw].rearrange("b p n -> p b n"),
                        in_=bgr[:, :, 0, oo : oo + nw],
                    )
                    nc.scalar.dma_start(
                        out=oi_d[c0 : c0 + 2, :, oo : oo + nw].rearrange("b p n -> p b n"),
                        in_=bgr[:, :, 1, oo : oo + nw],
                    )
                elif TRIANGLE and m in (1, 3):
                    lo = 0 if m == 1 else HALF_COL
                    hi = HALF_COL if m == 1 else TRI_W
                    nc.sync.dma_start(
                        out=or_d[c0 : c0 + 2, :, lo:hi].rearrange("b p n -> p b n"),
                        in_=bgr[:, :, 0, lo:hi],
                    )
                    nc.scalar.dma_start(
                        out=oi_d[c0 : c0 + 2, :, lo:hi].rearrange("b p n -> p b n"),
                        in_=bgr[:, :, 1, lo:hi],
                    )

            if not TRIANGLE:
                for jb, c in ((0, c0), (1, c0 + 1)):
                    nc.sync.dma_start(
                        out=or_d[c].rearrange("(m p) n -> p m n", p=128),
                        in_=bgr[:, jb, 0, :].rearrange("p (m n) -> p m n", n=D),
                    )
                    nc.scalar.dma_start(
                        out=oi_d[c].rearrange("(m p) n -> p m n", p=128),
                        in_=bgr[:, jb, 1, :].rearrange("p (m n) -> p m n", n=D),
                    )

    nc.compile()
    return nc


def _get_nc():
    if "nc" not in _CACHE:
        _CACHE["nc"] = _build_program()
    return _CACHE["nc"]


def _make_in_maps(input_real, input_imag, weight):
    np_in = np.float16 if PRECISION in ("fp16", "fp16o") else np.float32
    xcat = np.stack([input_real, input_imag], axis=2).astype(np_in)  # [B,T,2,D]
    in_maps = []
    for core in range(N_CORES):
        sl = slice(core * B_LOC, (core + 1) * B_LOC)
        wc = weight[sl]  # [B_LOC, T]
        wx = np.zeros((T, 54 * B_LOC), np.float32)
        for p in range(B_LOC // 2):
            we, wo = wc[2 * p], wc[2 * p + 1]
            be, bo = 108 * p, 108 * p + 6
            wx[:, be + 0] = we          # A  hot rel 0
            wx[:, be + 3] = we          # B  hot rel 1
            wx[:, be + 5] = -we         # D  hot rel 1
            wx[:, bo + 32] = wo         # A' hot rel 32
            wx[:, bo + 34 + 33] = wo    # B' hot rel 33
            wx[:, bo + 68 + 33] = -wo   # D' hot rel 33
        in_maps.append(
            {
                "xcat": np.ascontiguousarray(xcat[sl]),
                "wx": np.ascontiguousarray(wx, dtype=np_in),
            }
        )
    return in_maps


def _expand_tri(tri, sym):
    """tri: [B, 128, 1280] packed block-upper rows -> full [B, D, D].
    Chunk m holds rows [128m,128m+128) x cols [128m, D). Lower blocks are
    mirrored (sym=+1) or negated-mirrored (sym=-1)."""
    Bn = tri.shape[0]
    full = np.empty((Bn, D, D), dtype=np.float32)
    for m in range(4):
        rs = slice(m * 128, (m + 1) * 128)
        full[:, rs, m * 128 :] = tri[:, :, TRI_OFF[m] : TRI_OFF[m] + D - m * 128]
    for m in range(4):
        for n in range(m):
            full[:, m * 128 : (m + 1) * 128, n * 128 : (n + 1) * 128] = (
                sym * full[:, n * 128 : (n + 1) * 128, m * 128 : (m + 1) * 128]
                .transpose(0, 2, 1)
            )
    return full


def run(input_real, input_imag, weight, trace=False, **spmd_kwargs):
    """Build+run; returns (out_r, out_i, BassKernelResults)."""
    from concourse.bass_utils import run_bass_kernel_spmd

    input_real = np.asarray(input_real, dtype=np.float32)
    input_imag = np.asarray(input_imag, dtype=np.float32)
    weight = np.asarray(weight, dtype=np.float32)
    assert input_real.shape == (B, T, D), input_real.shape
    assert weight.shape == (B, T), weight.shape

    nc = _get_nc()
    in_maps = _make_in_maps(input_real, input_imag, weight)
    res = run_bass_kernel_spmd(
        nc, in_maps, list(range(N_CORES)), trace=trace, **spmd_kwargs
    )
    if TRIANGLE:
        tri_r = np.concatenate([np.asarray(r["out_r"]) for r in res.results], axis=0)
        # device "i" plane holds -out_i
        tri_i = -np.concatenate(
            [np.asarray(r["out_i"], dtype=np.float32) for r in res.results], axis=0
        )
        out_r = _expand_tri(tri_r, sym=1.0)
        out_i = _expand_tri(tri_i, sym=-1.0)
    else:
        out_r = np.concatenate(
            [np.asarray(r["out_r"], dtype=np.float32) for r in res.results], axis=0
        )
        out_i = -np.concatenate(
            [np.asarray(r["out_i"], dtype=np.float32) for r in res.results], axis=0
        )
    return out_r, out_i, res


def kernel(input_real, input_imag, weight):
    out_r, out_i, _ = run(input_real, input_imag, weight)
    return out_r, out_i
